# revision 37
# baseline (speedup 1.0000x reference)
"""Bass/Trainium2 kernel for nn_BoundaryAwareModule (KNN boundary-aware point module).

Algorithm per core (8 cores: batch b = core//4, query quarter q = core%4, Nq=2048):
  1. negdist [128q, N] via K=24 bf16 3-term (hi/mid/lo) matmul on PE:
     -dist = 2*q.a - |q|^2 - |a|^2, exact to ~2^-23 rel (full speed, 1 cyc/col)
  2. top-16 per row: segmented max8 (DVE) -> candidate merge -> max_index for
     indices; sqrt+row-sum for mean_d fused on ACT via accum_out
  3. per-k fp16 indirect gathers of neighbor features+xyz (264B rows; one
     offset per partition is the only layout hw handles correctly)
  4. max-over-k of features (DVE fp16 2x mode), sum-over-k of xyz (GPSIMD)
  5. small MLPs (boundary / spatial / attention) in fp16 on PE+ACT, folded BN
  6. out = x + boundary_feat * attn (fp32)

Emission is software-pipelined (PIPE_LAG): chunk i's gather-dependent tail
(trees/MLPs) is emitted after chunk i+1's scans so the in-order engine
queues don't head-of-line block on gather DMA completion.
Measured: 465,150 ns on 8 trn2 cores, rel err 9.96e-3 (gate 2e-2);
baseline was 852,357 ns.
"""
import numpy as np
import ml_dtypes

import concourse.bass as bass
import concourse.bacc as bacc
import concourse.mybir as mybir
import concourse.tile as tile
from concourse.bass_utils import run_bass_kernel_spmd

F32 = mybir.dt.float32
FP16 = mybir.dt.float16
BF16 = mybir.dt.bfloat16
I32 = mybir.dt.int32
U32 = mybir.dt.uint32
AX = mybir.AxisListType
ALU = mybir.AluOpType
ACTF = mybir.ActivationFunctionType

N = 8192          # points per batch
NQ = 2048         # queries per core
C = 128           # channels
K = 16            # neighbors
KD = 24           # contraction rows for the bf16 3-term distance matmul
SEG = 512         # segment size for pass-1 max8
GATHER_MODE = "per_k"   # "per_k" | "tposed"
GATHER_KG = 16           # k's per gather instruction in tposed mode
PIPE_LAG = 2             # chunks between head (scan+gather) and tail (MLP)
NEG = -3.0e38


def emit(tc, ins, outs, n=N, nq=NQ, seg=SEG, dbg=()):
    """Emit the per-core program. ins/outs: dicts name -> bass.AP (DRAM)."""
    NSEG = n // seg
    NCH = nq // 128
    nc = tc.nc
    import contextlib
    ctx = contextlib.ExitStack()
    with ctx:
        cpool = ctx.enter_context(tc.tile_pool(name="const", bufs=1))
        ndpool = ctx.enter_context(tc.tile_pool(name="nd", bufs=3))
        smpool = ctx.enter_context(tc.tile_pool(name="small", bufs=4))
        gpool = ctx.enter_context(tc.tile_pool(name="gather", bufs=5))
        mlppool = ctx.enter_context(tc.tile_pool(name="mlp", bufs=3))
        pdist = ctx.enter_context(tc.tile_pool(name="pdist", bufs=2, space="PSUM"))
        pmlp = ctx.enter_context(tc.tile_pool(name="pmlp", bufs=3, space="PSUM"))

        # ---- constants ----
        rhs24 = cpool.tile_from(ins["rhs24"])      # [KD, N] bf16
        lhsT24 = cpool.tile_from(ins["lhsT24"])    # [KD, NQ] bf16
        xq32 = cpool.tile_from(ins["xq32"])        # [C, NQ] f32
        xq16 = cpool.tile_from(ins["xq16"])        # [C, NQ] fp16
        xyzq4 = cpool.tile_from(ins["xyzq4"])      # [4, NQ] f32 (row 3 zeros)
        eye16 = cpool.tile_from(ins["eye16"])      # [128, 128] fp16
        eye32 = cpool.tile_from(ins["eye32"])      # [128, 128] f32
        segoff = cpool.tile_from(ins["segoff"])    # [128, NSEG*8] f32
        w1tx = cpool.tile_from(ins["w1tx"])
        w1td = cpool.tile_from(ins["w1td"])
        b1b = cpool.tile_from(ins["b1b"])
        w2t = cpool.tile_from(ins["w2t"])
        b2b = cpool.tile_from(ins["b2b"])
        a1tx = cpool.tile_from(ins["a1tx"])
        a1ts = cpool.tile_from(ins["a1ts"])
        ab1 = cpool.tile_from(ins["ab1"])
        a2t = cpool.tile_from(ins["a2t"])
        ab2 = cpool.tile_from(ins["ab2"])
        s1t4 = cpool.tile_from(ins["s1t4"])
        sb1 = cpool.tile_from(ins["sb1"])
        s2t = cpool.tile_from(ins["s2t"])
        sb2 = cpool.tile_from(ins["sb2"])
        out_d = outs["out"]

        def head(i):
            qsl = slice(i * 128, (i + 1) * 128)
            lhs_sl = lhsT24[:, qsl]                # [KD, 128]

            # ---- 1. negdist into SBUF via PE (bf16) + ACT copy ----
            nd = ndpool.tile([128, n], F32, tag="nd")
            for h in range(n // 1024):
                pd = pdist.tile([128, 1024], F32, tag="pd")
                c0 = h * 1024
                nc.tensor.matmul(pd[:, 0:512], lhs_sl, rhs24[:, c0:c0 + 512])
                nc.tensor.matmul(pd[:, 512:1024], lhs_sl, rhs24[:, c0 + 512:c0 + 1024])
                nc.scalar.copy(nd[:, c0:c0 + 1024], pd[:, :])

            # ---- 2. top-16 values + indices ----
            cand = smpool.tile([128, NSEG * 8], F32, tag="cand")
            for s in range(NSEG):
                nc.vector.max(cand[:, s * 8:(s + 1) * 8], nd[:, s * seg:(s + 1) * seg])
            candi = smpool.tile([128, NSEG * 8], U32, tag="candi")
            for s in range(NSEG):
                nc.vector.max_index(candi[:, s * 8:(s + 1) * 8],
                                    cand[:, s * 8:(s + 1) * 8],
                                    nd[:, s * seg:(s + 1) * seg])
            # global candidate indices (f32): candif = f32(candi) + seg*SEG
            candif = smpool.tile([128, NSEG * 8], F32, tag="candif")
            nc.vector.tensor_add(candif, candi, segoff)
            # merge candidate values -> top-16 values t16v (sorted desc)
            t16v = smpool.tile([128, 16], F32, tag="t16v")
            candr = smpool.tile([128, NSEG * 8], F32, tag="candr")
            nc.vector.max(t16v[:, 0:8], cand)
            nc.vector.match_replace(candr, t16v[:, 0:8], cand, NEG)
            nc.vector.max(t16v[:, 8:16], candr)
            # select candidates with value >= 16th value; among them take the
            # 16 smallest global indices (ties -> lowest index, matches jax)
            selm = smpool.tile([128, NSEG * 8], F32, tag="selm")
            nc.vector.tensor_scalar(selm, cand, t16v[:, 15:16], 1.0e9,
                                    op0=ALU.is_ge, op1=ALU.mult)
            mn = smpool.tile([128, NSEG * 8], F32, tag="mn")
            nc.vector.scalar_tensor_tensor(mn, in0=selm, scalar=-1.0e9,
                                           in1=candif, op0=ALU.add,
                                           op1=ALU.subtract)
            i16v = smpool.tile([128, 16], F32, tag="i16v")
            mnr = smpool.tile([128, NSEG * 8], F32, tag="mnr")
            nc.vector.max(i16v[:, 0:8], mn)
            nc.vector.match_replace(mnr, i16v[:, 0:8], mn, NEG)
            nc.vector.max(i16v[:, 8:16], mnr)

            # ---- mean_d = mean(sqrt(relu(-negdist))) fused on ACT; the
            # sqrt-sum lands directly in aux4 col 3 (f32 accum)
            aux4 = smpool.tile([128, 4], F32, tag="aux4")
            t16r = smpool.tile([128, 16], F32, tag="t16r")
            nc.scalar.activation(t16r, t16v, ACTF.Relu, scale=-1.0)
            d16s = smpool.tile([128, 16], FP16, tag="d16s")
            nc.scalar.activation(d16s, t16r, ACTF.Sqrt,
                                 accum_out=aux4[:, 3:4])

            # ---- 3. gather rows [x.T | xyz] fp16, 264B each ----
            gd = gpool.tile([128, K, 132], FP16, tag="gd")
            idxi = smpool.tile([128, 16], I32, tag="idxi")
            nc.vector.tensor_scalar_mul(idxi, i16v, -1.0)
            for k in range(K):
                nc.gpsimd.indirect_dma_start(
                    gd[:, k, :], None, ins["xtp"][:, :],
                    bass.IndirectOffsetOnAxis(ap=idxi[:, k:k + 1], axis=0))
            return dict(i=i, qsl=qsl, gd=gd, aux4=aux4, t16v=t16v,
                        candi=candi, candif=candif, i16v=i16v, idxi=idxi,
                        d16s=d16s)

        def tail(st):
            i, qsl, gd, aux4 = st["i"], st["qsl"], st["gd"], st["aux4"]
            xq16_sl = xq16[:, qsl]                 # [128, 128] fp16
            xq32_sl = xq32[:, qsl]                 # [128, 128] f32

            # ---- 4a. max over k of features (DVE fp16 2x) ----
            t1 = smpool.tile([128, 8, 128], FP16, tag="t1")
            t2 = smpool.tile([128, 4, 128], FP16, tag="t2")
            t3 = smpool.tile([128, 2, 128], FP16, tag="t3")
            gmax = smpool.tile([128, 1, 128], FP16, tag="gmax")
            nc.vector.tensor_max(t1, gd[:, 0:8, 0:128], gd[:, 8:16, 0:128])
            nc.vector.tensor_max(t2, t1[:, 0:4, :], t1[:, 4:8, :])
            nc.vector.tensor_max(t3, t2[:, 0:2, :], t2[:, 2:4, :])
            nc.vector.tensor_max(gmax, t3[:, 0:1, :], t3[:, 1:2, :])

            # ---- 4b. sum over k of xyz (GPSIMD fp16 tree -> f32 aux) ----
            u1 = smpool.tile([128, 8, 3], FP16, tag="u1")
            u2 = smpool.tile([128, 4, 3], FP16, tag="u2")
            u3 = smpool.tile([128, 2, 3], FP16, tag="u3")
            nc.gpsimd.tensor_add(u1, gd[:, 0:8, 128:131], gd[:, 8:16, 128:131])
            nc.gpsimd.tensor_add(u2, u1[:, 0:4, :], u1[:, 4:8, :])
            nc.gpsimd.tensor_add(u3, u2[:, 0:2, :], u2[:, 2:4, :])
            nc.gpsimd.tensor_add(aux4[:, 0:3], u3[:, 0, :], u3[:, 1, :])

            # ---- transposes to channel-major (PE, fp16) ----
            ptg = pmlp.tile([128, 128], FP16, tag="pm")
            nc.tensor.transpose(ptg, gmax[:, 0, :], eye16)
            pax = pmlp.tile([4, 128], F32, tag="pm")
            nc.tensor.transpose(pax, aux4, eye32)

            maxdiff = mlppool.tile([128, 128], FP16, tag="maxdiff")
            nc.vector.tensor_sub(maxdiff, ptg, xq16_sl)
            # spatial input: rows 0-2 mean_rel = sum_xyz/16 - q_xyz, row 3
            # mean_d = sum_d/16 - 0 (xyzq4 row 3 is zeros)
            spatial4 = mlppool.tile([4, 128], FP16, tag="spatial4")
            nc.vector.scalar_tensor_tensor(
                spatial4, in0=pax, scalar=1.0 / K,
                in1=xyzq4[:, qsl], op0=ALU.mult, op1=ALU.subtract,
            )

            # ---- 5. spatial MLP ----
            ps1 = pmlp.tile([32, 128], F32, tag="pm")
            nc.tensor.matmul(ps1, s1t4, spatial4)
            s1s = mlppool.tile([32, 128], FP16, tag="s1s")
            nc.scalar.activation(s1s, ps1, ACTF.Relu, bias=sb1[:, 0:1])
            ps2 = pmlp.tile([64, 128], F32, tag="pm")
            nc.tensor.matmul(ps2, s2t, s1s)
            spf = mlppool.tile([64, 128], FP16, tag="spf")
            nc.scalar.activation(spf, ps2, ACTF.Identity, bias=sb2[:, 0:1])

            # ---- attention MLP ----
            pa1 = pmlp.tile([64, 128], F32, tag="pm")
            nc.tensor.matmul(pa1, a1tx, xq16_sl, start=True, stop=False)
            nc.tensor.matmul(pa1, a1ts, spf, start=False, stop=True)
            a1s = mlppool.tile([64, 128], FP16, tag="a1s")
            nc.scalar.activation(a1s, pa1, ACTF.Relu, bias=ab1[:, 0:1])
            pa2 = pmlp.tile([128, 128], F32, tag="pm")
            nc.tensor.matmul(pa2, a2t, a1s)
            attn = mlppool.tile([128, 128], FP16, tag="attn")
            nc.scalar.activation(attn, pa2, ACTF.Sigmoid, bias=ab2[:, 0:1])

            # ---- boundary MLP ----
            pb1 = pmlp.tile([128, 128], F32, tag="pm")
            nc.tensor.matmul(pb1, w1tx, xq16_sl, start=True, stop=False)
            nc.tensor.matmul(pb1, w1td, maxdiff, start=False, stop=True)
            b1s = mlppool.tile([128, 128], FP16, tag="b1s")
            nc.scalar.activation(b1s, pb1, ACTF.Relu, bias=b1b[:, 0:1])
            pb2 = pmlp.tile([128, 128], F32, tag="pm")
            nc.tensor.matmul(pb2, w2t, b1s)
            b2s = mlppool.tile([128, 128], FP16, tag="b2s")
            nc.scalar.activation(b2s, pb2, ACTF.Relu, bias=b2b[:, 0:1])

            # ---- 6. out = x + boundary*attn ----
            oc = mlppool.tile([128, 128], F32, tag="oc")
            nc.vector.tensor_mul(oc, b2s, attn)
            nc.vector.tensor_add(oc, oc, xq32_sl)
            nc.sync.dma_start(out_d[:, qsl], oc)
            if i in (0, 1) and dbg:
                loc = dict(st)
                loc.update(locals())
                pre = "dbg_" if i == 0 else "dbg2_"
                for nm in dbg:
                    t = loc[nm]
                    nc.sync.dma_start(outs[pre + nm],
                                      t[:, :] if len(t.shape) == 2 else t[:, :, :].rearrange("p a b -> p (a b)"))

        pend = []
        for i in range(NCH):
            pend.append(head(i))
            if len(pend) > PIPE_LAG:
                tail(pend.pop(0))
        for st in pend:
            tail(st)


def _split3(v):
    """bf16 3-term decomposition: v ~= h + m + l to ~2^-24 rel."""
    b16 = ml_dtypes.bfloat16
    f32 = np.float32
    h = v.astype(b16).astype(f32)
    m = (v - h).astype(b16).astype(f32)
    l = (v - h - m).astype(b16).astype(f32)
    return h, m, l


def host_prep(x, xyz, bw1, bb1, bn1g, bn1b, bn1m, bn1v, bw2, bb2, bn2g, bn2b,
              bn2m, bn2v, sw1, sb1, sbng, sbnb, sbnm, sbnv, sw2, sb2,
              aw1, ab1, abng, abnb, abnm, abnv, aw2, ab2):
    """Fold BN into convs, build per-core input arrays. Returns list of 8 dicts."""
    f32 = np.float32
    f16 = np.float16
    b16 = ml_dtypes.bfloat16
    x = np.ascontiguousarray(x, f32)
    xyz = np.ascontiguousarray(xyz, f32)

    def fold(w, b, g, be, m, v):
        inv = (np.asarray(g, f32) / np.sqrt(np.asarray(v, f32) + 1e-5)).astype(f32)
        return (np.asarray(w, f32) * inv[:, None]).astype(f32), \
               (np.asarray(b, f32) * inv + np.asarray(be, f32) - np.asarray(m, f32) * inv).astype(f32)

    W1, B1 = fold(bw1, bb1, bn1g, bn1b, bn1m, bn1v)      # [128, 256]
    W2, B2 = fold(bw2, bb2, bn2g, bn2b, bn2m, bn2v)      # [128, 128]
    S1, SB1 = fold(sw1, sb1, sbng, sbnb, sbnm, sbnv)     # [32, 4]
    A1, AB1 = fold(aw1, ab1, abng, abnb, abnm, abnv)     # [64, 192]

    def col(v):
        return np.ascontiguousarray(np.asarray(v, f32).reshape(-1, 1))

    def t16(a):
        return np.ascontiguousarray(np.asarray(a, f32).astype(f16))

    seg_off_row = np.repeat(np.arange(N // SEG, dtype=f32) * SEG, 8)
    consts = {
        "eye16": np.eye(128, dtype=f16),
        "eye32": np.eye(128, dtype=f32),
        "segoff": np.ascontiguousarray(np.broadcast_to(seg_off_row, (128, seg_off_row.size))),
        "w1tx": t16(W1[:, :128].T),
        "w1td": t16(W1[:, 128:].T),
        "b1b": col(B1),
        "w2t": t16(W2.T),
        "b2b": col(B2),
        "a1tx": t16(A1[:, :128].T),
        "a1ts": t16(A1[:, 128:192].T),
        "ab1": col(AB1),
        "a2t": t16(np.asarray(aw2, f32).T),
        "ab2": col(ab2),
        "s1t4": t16(S1.T),                               # [4, 32]
        "sb1": col(SB1),
        "s2t": t16(np.asarray(sw2, f32).T),
        "sb2": col(sb2),
    }

    per_batch = []
    for b in range(2):
        xx = (xyz[b].astype(np.float64) ** 2).sum(1).astype(f32)   # [N]
        ah, am, al = _split3(xyz[b].T)                   # [3, N] each
        nxh, nxm, nxl = _split3(-xx)                     # [N] each
        ones = np.ones((1, N), f32)
        # rhs rows matched with lhsT rows (see below)
        rhs24 = np.concatenate([
            ah, am, ah, al, ah, am,                      # rows 0-17
            ones, ones, ones,                            # rows 18-20 (q norm)
            nxh[None, :], nxm[None, :], nxl[None, :],    # rows 21-23 (a norm)
        ], 0).astype(b16)
        xtp = np.zeros((N, 132), f16)
        xtp[:, :128] = x[b].T.astype(f16)
        xtp[:, 128:131] = xyz[b].astype(f16)
        per_batch.append((xx, rhs24, xtp))

    in_maps = []
    for core in range(8):
        b, q = core // 4, core % 4
        sl = slice(q * NQ, (q + 1) * NQ)
        xx, rhs24, xtp = per_batch[b]
        qh, qm, ql = _split3(2.0 * xyz[b][sl].T)         # [3, NQ]
        nqh, nqm, nql = _split3(-xx[sl])                 # [NQ]
        onesq = np.ones((1, NQ), f32)
        lhsT24 = np.concatenate([
            qh, qh, qm, qh, ql, qm,                      # rows 0-17
            nqh[None, :], nqm[None, :], nql[None, :],    # rows 18-20
            onesq, onesq, onesq,                         # rows 21-23
        ], 0).astype(b16)
        m = dict(consts)
        m["rhs24"] = rhs24
        m["lhsT24"] = np.ascontiguousarray(lhsT24)
        m["xq32"] = np.ascontiguousarray(x[b][:, sl])
        m["xq16"] = np.ascontiguousarray(x[b][:, sl].astype(f16))
        m["xyzq4"] = np.ascontiguousarray(
            np.concatenate([xyz[b][sl].T, np.zeros((1, NQ), f32)], 0))
        m["xtp"] = xtp
        in_maps.append(m)
    return in_maps


def _specs(n=N, nq=NQ, seg=SEG):
  return {
    "rhs24": ([KD, n], BF16), "lhsT24": ([KD, nq], BF16),
    "xq32": ([C, nq], F32), "xq16": ([C, nq], FP16),
    "xyzq4": ([4, nq], F32), "xtp": ([n, 132], FP16),
    "eye16": ([128, 128], FP16), "eye32": ([128, 128], F32),
    "segoff": ([128, (n // seg) * 8], F32),
    "w1tx": ([128, 128], FP16), "w1td": ([128, 128], FP16), "b1b": ([128, 1], F32),
    "w2t": ([128, 128], FP16), "b2b": ([128, 1], F32),
    "a1tx": ([128, 64], FP16), "a1ts": ([64, 64], FP16), "ab1": ([64, 1], F32),
    "a2t": ([64, 128], FP16), "ab2": ([128, 1], F32),
    "s1t4": ([4, 32], FP16), "sb1": ([32, 1], F32),
    "s2t": ([32, 64], FP16), "sb2": ([64, 1], F32),
  }

_CACHE = {}


def build_program(n=N, nq=NQ, seg=SEG, dbg=()):
    key = (n, nq, seg, tuple(dbg), GATHER_MODE, GATHER_KG)
    if key in _CACHE:
        return _CACHE[key]
    nc = bacc.Bacc("TRN2", debug=False, num_devices=8)
    ins = {name: nc.dram_tensor(name, shape, dt, kind="ExternalInput").ap()
           for name, (shape, dt) in _specs(n, nq, seg).items()}
    outs = {"out": nc.dram_tensor("out", [C, nq], F32, kind="ExternalOutput").ap()}
    if dbg:
        import functools, operator
        dbg_shapes = {
            "nd": ([128, n], F32), "cand": ([128, 128], F32),
            "candi": ([128, 128], U32), "candif": ([128, 128], F32),
            "t16v": ([128, 16], F32), "i16v": ([128, 16], F32),
            "idxi": ([128, 16], I32), "idxiT": ([16, 128], I32),
            "gd": ([128, K * 132], FP16),
            "gmax": ([128, 128], FP16),
            "maxdiff": ([128, 128], FP16), "spatial4": ([4, 128], FP16),
            "attn": ([128, 128], FP16), "b2s": ([128, 128], FP16),
            "aux4": ([128, 4], F32), "d16s": ([128, 16], FP16),
        }
        for pre in ("dbg_", "dbg2_"):
            for nm in dbg:
                sh, dt = dbg_shapes[nm]
                outs[pre + nm] = nc.dram_tensor(pre + nm, sh, dt,
                                                kind="ExternalOutput").ap()
    with tile.TileContext(nc) as tc:
        emit(tc, ins, outs, n, nq, seg, dbg=dbg)
    nc.compile()
    _CACHE[key] = nc
    return nc


def kernel(**inputs):
    in_maps = host_prep(**inputs)
    nc = build_program()
    res = run_bass_kernel_spmd(nc, in_maps, core_ids=list(range(8)))
    _CACHE["last_results"] = res
    out = np.empty((2, C, N), np.float32)
    for core in range(8):
        b, q = core // 4, core % 4
        out[b][:, q * NQ:(q + 1) * NQ] = res.results[core]["out"]
    return out


if __name__ == "__main__":
    import reference
    inputs = {k: np.asarray(v) for k, v in reference.setup_inputs().items()}
    got = kernel(**inputs)
    exp = np.asarray(reference.reference(**inputs))
    err = np.abs(got - exp)
    print("absmax err:", err.max(), "rel:", err.max() / np.abs(exp).max())
